# revision 1
# baseline (speedup 1.0000x reference)
"""Trainium2 Bass kernel for nn_Conv_39273180955616.

Computes, for X:(16,64,512,512) f32, K:(1,1,7,7), b:(1,1,1,1):
    out[n,c] = correlate2d(X[n,c], Keff, pad=3) + 49*b
where Keff = K.sum(axis=(0,1)).

Strategy: pure data parallel over the 1024 (n,c) planes -> 128 planes/core
on 8 cores.  Per plane, the 7x7 correlation is computed on TensorE as
banded-Toeplitz matmuls: the h-dimension contraction is a [K<=128, M]
band matrix (7 diagonals of one kernel column) against the image block
(rows on partitions), and the 7 w-shifts are free-dim offsets into a
zero-padded (W+6) image row, accumulated in PSUM.  Inputs are pre-cast
to bf16 on host (PSUM accumulates in fp32); bias is added on ScalarE
during PSUM->SBUF eviction.
"""
import numpy as np
import ml_dtypes

import concourse.bass as bass
import concourse.tile as tile
from concourse import bacc, mybir
from concourse.bass_utils import run_bass_kernel_spmd

N_CORES = 8
H = 512
W = 512
WPAD = W + 6  # 3 zero columns each side
N_PLANES_TOTAL = 16 * 64
PLANES_PER_CORE = N_PLANES_TOTAL // N_CORES  # 128

# (out_row0, out_rows, in_row0, in_rows, kind)
# kind 0: top boundary (K=128, M=125), band dh = p - m + 3
# kind 1: interior     (K=128, M=122), band dh = p - m
# kind 2: bottom       (K=24,  M=21),  band dh = p - m
TILES = [
    (0, 125, 0, 128, 0),
    (125, 122, 122, 128, 1),
    (247, 122, 244, 128, 1),
    (369, 122, 366, 128, 1),
    (491, 21, 488, 24, 2),
]
KIND_KM = {0: (128, 125), 1: (128, 122), 2: (24, 21)}
# column offsets of each kind's 7 matrices in the packed weight tensor
WOFF = {}
_c = 0
for _k in (0, 1, 2):
    WOFF[_k] = _c
    _c += 7 * KIND_KM[_k][1]
WCOLS = _c  # 7*(125+122+21) = 1876


def _build_weight_pack(Keff: np.ndarray) -> np.ndarray:
    """Keff: (7,7) f32 -> packed banded-Toeplitz lhsT matrices, [128, WCOLS] bf16.

    lhsT[p, m] = Keff[dh, dw] with dh = p - m (+3 for the top tile); matmul
    computes out[m, w] = sum_p lhsT[p, m] * block[p, w + dw], i.e. the
    h-contraction of the correlation for kernel column dw.
    """
    wp = np.zeros((128, WCOLS), np.float32)
    dh_off = {0: 3, 1: 0, 2: 0}
    for kind, (Kk, Mk) in KIND_KM.items():
        off = WOFF[kind]
        p = np.arange(Kk)[:, None]
        m = np.arange(Mk)[None, :]
        dh = p - m + dh_off[kind]
        valid = (dh >= 0) & (dh < 7)
        for dw in range(7):
            mat = np.zeros((Kk, Mk), np.float32)
            mat[valid] = Keff[dh[valid], dw]
            wp[:Kk, off + dw * Mk: off + dw * Mk + Mk] = mat
    return wp.astype(ml_dtypes.bfloat16)


_NC_CACHE = {}


def _get_module(n_planes: int):
    if n_planes in _NC_CACHE:
        return _NC_CACHE[n_planes]
    nc = bacc.Bacc("TRN2", target_bir_lowering=False, debug=False,
                   num_devices=N_CORES)
    xp = nc.dram_tensor("xp", [n_planes, H, WPAD], mybir.dt.bfloat16,
                        kind="ExternalInput")
    wt = nc.dram_tensor("wt", [128, WCOLS], mybir.dt.bfloat16,
                        kind="ExternalInput")
    bv = nc.dram_tensor("bv", [128, 1], mybir.dt.float32,
                        kind="ExternalInput")
    out = nc.dram_tensor("out", [n_planes, H, W], mybir.dt.float32,
                         kind="ExternalOutput")

    with tile.TileContext(nc) as tc:
        with (
            tc.tile_pool(name="wp", bufs=1) as wpool,
            tc.tile_pool(name="xp", bufs=6) as xpool,
            tc.tile_pool(name="ps", bufs=8, space="PSUM") as pspool,
            tc.tile_pool(name="op", bufs=6) as opool,
        ):
            wtile = wpool.tile([128, WCOLS], mybir.dt.bfloat16)
            nc.sync.dma_start(wtile[:], wt.ap())
            btile = wpool.tile([128, 1], mybir.dt.float32)
            nc.sync.dma_start(btile[:], bv.ap())
            xap = xp.ap()
            oap = out.ap()
            for p in range(n_planes):
                for (or0, oh, ir0, ih, kind) in TILES:
                    xt = xpool.tile([128, WPAD], mybir.dt.bfloat16)
                    nc.sync.dma_start(xt[:ih, :], xap[p, ir0:ir0 + ih, :])
                    pt = pspool.tile([128, W], mybir.dt.float32)
                    off = WOFF[kind]
                    M = KIND_KM[kind][1]
                    for dw in range(7):
                        nc.tensor.matmul(
                            pt[:oh, :],
                            wtile[:ih, off + dw * M: off + dw * M + M],
                            xt[:ih, dw:dw + W],
                            start=(dw == 0), stop=(dw == 6),
                        )
                    ot = opool.tile([128, W], mybir.dt.float32)
                    nc.scalar.activation(
                        ot[:oh, :], pt[:oh, :],
                        mybir.ActivationFunctionType.Identity,
                        bias=btile[:oh, :], scale=1.0,
                    )
                    nc.sync.dma_start(oap[p, or0:or0 + oh, :], ot[:oh, :])

    nc.compile()
    _NC_CACHE[n_planes] = nc
    return nc


def _prep_inputs(X, K, b, n_cores=N_CORES):
    Keff = np.asarray(K, np.float32).sum(axis=(0, 1))
    wt = _build_weight_pack(Keff)
    bias = np.float32(np.asarray(b).reshape(-1)[0]) * np.float32(K.size)
    bv = np.full((128, 1), bias, np.float32)

    Xr = np.asarray(X, np.float32).reshape(-1, H, W)
    n_total = Xr.shape[0]
    per = n_total // n_cores
    Xp = np.zeros((n_total, H, WPAD), ml_dtypes.bfloat16)
    Xp[:, :, 3:3 + W] = Xr.astype(ml_dtypes.bfloat16)
    in_maps = [
        {"xp": Xp[i * per:(i + 1) * per], "wt": wt, "bv": bv}
        for i in range(n_cores)
    ]
    return in_maps, per


def kernel(X, K, b):
    in_maps, per = _prep_inputs(X, K, b)
    nc = _get_module(per)
    res = run_bass_kernel_spmd(nc, in_maps, list(range(N_CORES)))
    out = np.concatenate([res.results[i]["out"] for i in range(N_CORES)], axis=0)
    return out.reshape(np.asarray(X).shape)


# revision 2
# speedup vs baseline: 1.5241x; 1.5241x over previous
"""Trainium2 Bass kernel for nn_Conv_39273180955616.

Computes, for X:(16,64,512,512) f32, K:(1,1,7,7), b:(1,1,1,1):
    out[n,c] = correlate2d(X[n,c], Keff, pad=3) + 49*b
where Keff = K.sum(axis=(0,1)).

Strategy: pure data parallel over the 1024 (n,c) planes -> 128 planes/core
on 8 cores.  Per plane, the 7x7 correlation runs on TensorE as
banded-Toeplitz matmuls: the h-dimension contraction is a [K<=128, 128]
band matrix (7 diagonals of one kernel column) against an image block
(rows on partitions), and the 7 w-shifts are free-dim offsets into a
zero-padded (W+6) image row, accumulated in PSUM.  Inputs are pre-cast
to bf16 on host (PSUM accumulates in fp32); bias is added on ScalarE
during PSUM->SBUF eviction.  DMA is batched per plane (3 loads / 2
stores) and spread across the SP-HWDGE, ACT-HWDGE and SWDGE rings.
"""
import numpy as np
import ml_dtypes

import concourse.bass as bass
import concourse.tile as tile
from concourse import bacc, mybir
from concourse.bass_utils import run_bass_kernel_spmd

N_CORES = 8
H = 512
W = 512
WPAD = W + 6  # 3 zero columns each side
N_PLANES_TOTAL = 16 * 64
PLANES_PER_CORE = N_PLANES_TOTAL // N_CORES  # 128

# Output tiles: 4 x 122 rows (batched, affine) + 1 x 24 rows.
# (out_row0, out_rows, in_row0, in_rows, kind)
# kind 0: top      K=125 (rows  0..124), band dh = p - m + 3
# kind 1: interior K=128 (rows or0-3..), band dh = p - m
# kind 2: bottom   K=27  (rows 485..511), band dh = p - m
TILES = [
    (0, 122, 0, 125, 0),
    (122, 122, 119, 128, 1),
    (244, 122, 241, 128, 1),
    (366, 122, 363, 128, 1),
    (488, 24, 485, 27, 2),
]
KIND_K = {0: 125, 1: 128, 2: 27}
M_PAD = 128  # lhsT padded to 128 columns -> FWL eligible, zero pad rows in PSUM
WCOLS = 3 * 7 * M_PAD


def _build_weight_pack(Keff: np.ndarray) -> np.ndarray:
    """Keff (7,7) f32 -> packed banded-Toeplitz lhsT matrices [128, WCOLS] bf16.

    Matrix for (kind, dw) sits at cols [(kind*7+dw)*128, ...+128).
    lhsT[p, m] = Keff[dh, dw], dh = p - m (+3 for kind 0); matmul then gives
    out[m, w] = sum_p lhsT[p, m] * block[p, w + dw].
    """
    wp = np.zeros((128, WCOLS), np.float32)
    dh_off = {0: 3, 1: 0, 2: 0}
    valid_m = {0: 122, 1: 122, 2: 24}
    for kind in (0, 1, 2):
        Kk = KIND_K[kind]
        Mk = valid_m[kind]
        p = np.arange(Kk)[:, None]
        m = np.arange(Mk)[None, :]
        dh = p - m + dh_off[kind]
        ok = (dh >= 0) & (dh < 7)
        for dw in range(7):
            mat = np.zeros((Kk, M_PAD), np.float32)
            mat[:, :Mk][ok] = Keff[dh[ok], dw]
            c0 = (kind * 7 + dw) * M_PAD
            wp[:Kk, c0:c0 + M_PAD] = mat
    return wp.astype(ml_dtypes.bfloat16)


_NC_CACHE = {}


def _get_module(n_planes: int):
    if n_planes in _NC_CACHE:
        return _NC_CACHE[n_planes]
    nc = bacc.Bacc("TRN2", target_bir_lowering=False, debug=False,
                   num_devices=N_CORES)
    xp = nc.dram_tensor("xp", [n_planes, H, WPAD], mybir.dt.bfloat16,
                        kind="ExternalInput")
    wt = nc.dram_tensor("wt", [128, WCOLS], mybir.dt.bfloat16,
                        kind="ExternalInput")
    bv = nc.dram_tensor("bv", [128, 1], mybir.dt.float32,
                        kind="ExternalInput")
    out = nc.dram_tensor("out", [n_planes, H, W], mybir.dt.float32,
                         kind="ExternalOutput")

    x_elems = H * WPAD  # per-plane element count in xp

    with tile.TileContext(nc) as tc:
        with (
            tc.tile_pool(name="wp", bufs=1) as wpool,
            tc.tile_pool(name="xa", bufs=4) as xapool,
            tc.tile_pool(name="xb", bufs=4) as xbpool,
            tc.tile_pool(name="xc", bufs=4) as xcpool,
            tc.tile_pool(name="ps", bufs=8, space="PSUM") as pspool,
            tc.tile_pool(name="ob", bufs=3) as obpool,
            tc.tile_pool(name="oc", bufs=3) as ocpool,
        ):
            wtile = wpool.tile([128, WCOLS], mybir.dt.bfloat16)
            nc.sync.dma_start(wtile[:], wt.ap())
            btile = wpool.tile([128, 1], mybir.dt.float32)
            nc.sync.dma_start(btile[:], bv.ap())
            oap = out.ap()
            for p in range(n_planes):
                # ---- input loads (SP ring) ----
                xa = xapool.tile([125, WPAD], mybir.dt.bfloat16)  # rows 0..124
                nc.sync.dma_start(
                    xa[:], bass.AP(xp, p * x_elems,
                                   [[WPAD, 125], [1, WPAD]]))
                xb = xbpool.tile([128, 3 * WPAD], mybir.dt.bfloat16)
                # rows 119+122b+p, b=0..2 (overlapping strided read)
                nc.sync.dma_start(
                    xb[:].rearrange("p (b w) -> p b w", b=3),
                    bass.AP(xp, p * x_elems + 119 * WPAD,
                            [[WPAD, 128], [122 * WPAD, 3], [1, WPAD]]))
                xc = xcpool.tile([27, WPAD], mybir.dt.bfloat16)  # rows 485..511
                nc.sync.dma_start(
                    xc[:], bass.AP(xp, p * x_elems + 485 * WPAD,
                                   [[WPAD, 27], [1, WPAD]]))

                ob = obpool.tile([122, 4 * W], mybir.dt.float32)
                oc = ocpool.tile([24, W], mybir.dt.float32)
                for t, (or0, oh, ir0, ih, kind) in enumerate(TILES):
                    if kind == 0:
                        rhs_of = lambda dw: xa[:, dw:dw + W]
                    elif kind == 2:
                        rhs_of = lambda dw: xc[:, dw:dw + W]
                    else:
                        b = t - 1
                        rhs_of = lambda dw, b=b: xb[:, b * WPAD + dw:
                                                    b * WPAD + dw + W]
                    pt = pspool.tile([128, W], mybir.dt.float32)
                    for dw in range(7):
                        c0 = (kind * 7 + dw) * M_PAD
                        nc.tensor.matmul(
                            pt[:, :], wtile[:ih, c0:c0 + M_PAD], rhs_of(dw),
                            start=(dw == 0), stop=(dw == 6),
                        )
                    # eviction + bias (ACT)
                    if t < 4:
                        nc.scalar.activation(
                            ob[:, t * W:(t + 1) * W], pt[:122, :],
                            mybir.ActivationFunctionType.Identity,
                            bias=btile[:122, :], scale=1.0)
                    else:
                        nc.scalar.activation(
                            oc[:], pt[:24, :],
                            mybir.ActivationFunctionType.Identity,
                            bias=btile[:24, :], scale=1.0)
                # ---- stores ----
                # rows 0..487 = 4 tiles of 122 (SWDGE ring, 1 MB)
                nc.gpsimd.dma_start(
                    bass.AP(out, p * H * W, [[W, 122], [122 * W, 4], [1, W]]),
                    ob[:].rearrange("p (b w) -> p b w", b=4))
                # rows 488..511 (ACT ring, small)
                nc.scalar.dma_start(
                    bass.AP(out, (p * H + 488) * W, [[W, 24], [1, W]]),
                    oc[:])

    nc.compile()
    _NC_CACHE[n_planes] = nc
    return nc


def _prep_inputs(X, K, b, n_cores=N_CORES):
    Keff = np.asarray(K, np.float32).sum(axis=(0, 1))
    wt = _build_weight_pack(Keff)
    bias = np.float32(np.asarray(b).reshape(-1)[0]) * np.float32(K.size)
    bv = np.full((128, 1), bias, np.float32)

    Xr = np.asarray(X, np.float32).reshape(-1, H, W)
    n_total = Xr.shape[0]
    per = n_total // n_cores
    Xp = np.zeros((n_total, H, WPAD), ml_dtypes.bfloat16)
    Xp[:, :, 3:3 + W] = Xr.astype(ml_dtypes.bfloat16)
    in_maps = [
        {"xp": Xp[i * per:(i + 1) * per], "wt": wt, "bv": bv}
        for i in range(n_cores)
    ]
    return in_maps, per


def kernel(X, K, b):
    in_maps, per = _prep_inputs(X, K, b)
    nc = _get_module(per)
    res = run_bass_kernel_spmd(nc, in_maps, list(range(N_CORES)))
    out = np.concatenate([res.results[i]["out"] for i in range(N_CORES)], axis=0)
    return out.reshape(np.asarray(X).shape)


# revision 3
# speedup vs baseline: 1.9539x; 1.2821x over previous
"""Trainium2 Bass kernel for nn_Conv_39273180955616.

Computes, for X:(16,64,512,512) f32, K:(1,1,7,7), b:(1,1,1,1):
    out[n,c] = correlate2d(X[n,c], Keff, pad=3) + 49*b
where Keff = K.sum(axis=(0,1)).

Strategy: pure data parallel over the 1024 (n,c) planes -> 128 planes/core
on 8 cores.  Per plane, the 7x7 correlation runs on TensorE as
banded-Toeplitz matmuls: the h-dimension contraction is a [K<=128, 128]
band matrix (7 diagonals of one kernel column) against an image block
(rows on partitions), and the 7 w-shifts are free-dim offsets into a
zero-padded (W+6) image row, accumulated in PSUM.  Inputs are pre-cast
to bf16 on host (PSUM accumulates in fp32); bias is added on ScalarE
during PSUM->SBUF eviction.  DMA is batched per plane (3 loads / 2
stores) and spread across the SP-HWDGE, ACT-HWDGE and SWDGE rings.
"""
import numpy as np
import ml_dtypes

import concourse.bass as bass
import concourse.tile as tile
from concourse import bacc, mybir
from concourse.bass_utils import run_bass_kernel_spmd

N_CORES = 8
H = 512
W = 512
WPAD = W + 6  # 3 zero columns each side
N_PLANES_TOTAL = 16 * 64
PLANES_PER_CORE = N_PLANES_TOTAL // N_CORES  # 128

# Output tiles: 4 x 122 rows (batched, affine) + 1 x 24 rows.
# (out_row0, out_rows, in_row0, in_rows, kind)
# kind 0: top      K=125 (rows  0..124), band dh = p - m + 3
# kind 1: interior K=128 (rows or0-3..), band dh = p - m
# kind 2: bottom   K=27  (rows 485..511), band dh = p - m
TILES = [
    (0, 122, 0, 125, 0),
    (122, 122, 119, 128, 1),
    (244, 122, 241, 128, 1),
    (366, 122, 363, 128, 1),
    (488, 24, 485, 27, 2),
]
KIND_K = {0: 125, 1: 128, 2: 27}
M_PAD = 128  # lhsT padded to 128 columns -> FWL eligible, zero pad rows in PSUM
WCOLS = 3 * 7 * M_PAD


def _build_weight_pack(Keff: np.ndarray) -> np.ndarray:
    """Keff (7,7) f32 -> packed banded-Toeplitz lhsT matrices [128, WCOLS] bf16.

    Matrix for (kind, dw) sits at cols [(kind*7+dw)*128, ...+128).
    lhsT[p, m] = Keff[dh, dw], dh = p - m (+3 for kind 0); matmul then gives
    out[m, w] = sum_p lhsT[p, m] * block[p, w + dw].
    """
    wp = np.zeros((128, WCOLS), np.float32)
    dh_off = {0: 3, 1: 0, 2: 0}
    valid_m = {0: 122, 1: 122, 2: 24}
    for kind in (0, 1, 2):
        Kk = KIND_K[kind]
        Mk = valid_m[kind]
        p = np.arange(Kk)[:, None]
        m = np.arange(Mk)[None, :]
        dh = p - m + dh_off[kind]
        ok = (dh >= 0) & (dh < 7)
        for dw in range(7):
            mat = np.zeros((Kk, M_PAD), np.float32)
            mat[:, :Mk][ok] = Keff[dh[ok], dw]
            c0 = (kind * 7 + dw) * M_PAD
            wp[:Kk, c0:c0 + M_PAD] = mat
    return wp.astype(ml_dtypes.bfloat16)


_NC_CACHE = {}


def _get_module(n_planes: int):
    if n_planes in _NC_CACHE:
        return _NC_CACHE[n_planes]
    nc = bacc.Bacc("TRN2", target_bir_lowering=False, debug=False,
                   num_devices=N_CORES)
    xp = nc.dram_tensor("xp", [n_planes, H, WPAD], mybir.dt.bfloat16,
                        kind="ExternalInput")
    wt = nc.dram_tensor("wt", [128, WCOLS], mybir.dt.bfloat16,
                        kind="ExternalInput")
    bv = nc.dram_tensor("bv", [128, 1], mybir.dt.float32,
                        kind="ExternalInput")
    out = nc.dram_tensor("out", [n_planes, H, W], mybir.dt.float32,
                         kind="ExternalOutput")

    x_elems = H * WPAD  # per-plane element count in xp

    with tile.TileContext(nc) as tc:
        with (
            tc.tile_pool(name="wp", bufs=1) as wpool,
            tc.tile_pool(name="xa", bufs=6) as xapool,
            tc.tile_pool(name="xb", bufs=6) as xbpool,
            tc.tile_pool(name="xc", bufs=6) as xcpool,
            tc.tile_pool(name="ps", bufs=8, space="PSUM") as pspool,
            tc.tile_pool(name="ob", bufs=8) as obpool,
            tc.tile_pool(name="oc", bufs=6) as ocpool,
        ):
            wtile = wpool.tile([128, WCOLS], mybir.dt.bfloat16)
            nc.sync.dma_start(wtile[:], wt.ap())
            btile = wpool.tile([128, 1], mybir.dt.float32)
            nc.sync.dma_start(btile[:], bv.ap())
            oap = out.ap()
            for p in range(n_planes):
                # ---- input loads (SP ring) ----
                xa = xapool.tile([125, WPAD], mybir.dt.bfloat16)  # rows 0..124
                nc.sync.dma_start(
                    xa[:], bass.AP(xp, p * x_elems,
                                   [[WPAD, 125], [1, WPAD]]))
                xb = xbpool.tile([128, 3 * WPAD], mybir.dt.bfloat16)
                # rows 119+122b+p, b=0..2 (overlapping strided read)
                nc.sync.dma_start(
                    xb[:].rearrange("p (b w) -> p b w", b=3),
                    bass.AP(xp, p * x_elems + 119 * WPAD,
                            [[WPAD, 128], [122 * WPAD, 3], [1, WPAD]]))
                xc = xcpool.tile([27, WPAD], mybir.dt.bfloat16)  # rows 485..511
                nc.sync.dma_start(
                    xc[:], bass.AP(xp, p * x_elems + 485 * WPAD,
                                   [[WPAD, 27], [1, WPAD]]))

                ob = obpool.tile([122, 4 * W], mybir.dt.float32)
                oc = ocpool.tile([24, W], mybir.dt.float32)
                for t, (or0, oh, ir0, ih, kind) in enumerate(TILES):
                    if kind == 0:
                        rhs_of = lambda dw: xa[:, dw:dw + W]
                    elif kind == 2:
                        rhs_of = lambda dw: xc[:, dw:dw + W]
                    else:
                        b = t - 1
                        rhs_of = lambda dw, b=b: xb[:, b * WPAD + dw:
                                                    b * WPAD + dw + W]
                    pt = pspool.tile([128, W], mybir.dt.float32)
                    for dw in range(7):
                        c0 = (kind * 7 + dw) * M_PAD
                        nc.tensor.matmul(
                            pt[:, :], wtile[:ih, c0:c0 + M_PAD], rhs_of(dw),
                            start=(dw == 0), stop=(dw == 6),
                        )
                    # eviction + bias (ACT)
                    if t < 4:
                        nc.scalar.activation(
                            ob[:, t * W:(t + 1) * W], pt[:122, :],
                            mybir.ActivationFunctionType.Identity,
                            bias=btile[:122, :], scale=1.0)
                    else:
                        nc.scalar.activation(
                            oc[:], pt[:24, :],
                            mybir.ActivationFunctionType.Identity,
                            bias=btile[:24, :], scale=1.0)
                # ---- stores ----
                # rows 0..487 = 4 tiles of 122 (SWDGE ring, 1 MB)
                nc.gpsimd.dma_start(
                    bass.AP(out, p * H * W, [[W, 122], [122 * W, 4], [1, W]]),
                    ob[:].rearrange("p (b w) -> p b w", b=4))
                # rows 488..511 (SP ring, small; keep ACT free of DMA dispatch)
                nc.sync.dma_start(
                    bass.AP(out, (p * H + 488) * W, [[W, 24], [1, W]]),
                    oc[:])

    nc.compile()
    _NC_CACHE[n_planes] = nc
    return nc


def _prep_inputs(X, K, b, n_cores=N_CORES):
    Keff = np.asarray(K, np.float32).sum(axis=(0, 1))
    wt = _build_weight_pack(Keff)
    bias = np.float32(np.asarray(b).reshape(-1)[0]) * np.float32(K.size)
    bv = np.full((128, 1), bias, np.float32)

    Xr = np.asarray(X, np.float32).reshape(-1, H, W)
    n_total = Xr.shape[0]
    per = n_total // n_cores
    Xp = np.zeros((n_total, H, WPAD), ml_dtypes.bfloat16)
    Xp[:, :, 3:3 + W] = Xr.astype(ml_dtypes.bfloat16)
    in_maps = [
        {"xp": Xp[i * per:(i + 1) * per], "wt": wt, "bv": bv}
        for i in range(n_cores)
    ]
    return in_maps, per


def kernel(X, K, b):
    in_maps, per = _prep_inputs(X, K, b)
    nc = _get_module(per)
    res = run_bass_kernel_spmd(nc, in_maps, list(range(N_CORES)))
    out = np.concatenate([res.results[i]["out"] for i in range(N_CORES)], axis=0)
    return out.reshape(np.asarray(X).shape)
